# revision 1
# baseline (speedup 1.0000x reference)
"""AdjMultiHeadAttention Trainium2 kernel.

Sharding: pure data-parallel over batch. B=16 batches over 8 NeuronCores
-> 2 batches per core. Weights replicated. No collectives.

Per-core algorithm (all compute in bf16 on the TensorEngine, fp32 PSUM):
  - host pre-transposes x -> xT [E,S] and mask=(adj+bond) -> maskT [sk,sq],
    pre-transposes weights, folds the 1/sqrt(d) scale into Wq.
  - q/k projections produce qT/kT [f, s] directly (contraction over e).
  - v projection produces v natural [s, f] (x chunks as stationary).
  - scores are computed transposed: scoresT[sk,sq], so softmax's reduction
    axis lands on PSUM partitions and the ctx matmul can consume attn^T
    directly as the stationary operand.
  - mask multiply is fused into the single PSUM->SBUF pass on the vector
    engine (scalar_tensor_tensor); exp runs as quarter-head [128, 2048]
    activations on the scalar engine (finer ops avoid head-of-line
    blocking in the in-order ACT stream).
  - softmax denominator comes from a ones-column appended to v (65th
    column of each head's v block) accumulated in the same ctx matmul.
  - ctx rows are scaled by 1/denom (+bv) while draining PSUM, transposed
    128x64-block-wise on the TensorEngine, and fed to the out projection.

Emission is software-pipelined over units u = (batch, head-pair):
projections are prefetched two units ahead so the vector engine (the
bottleneck: 16.8M PSUM mask-multiply reads at 1 elem/lane/cycle) never
starves.
"""

import sys

sys.path.insert(0, "/opt/trn_rl_repo")

from contextlib import ExitStack

import ml_dtypes
import numpy as np

import concourse.bass as bass
import concourse.tile as tile
from concourse import bacc, mybir
from concourse.bass_utils import run_bass_kernel_spmd
from concourse.masks import make_identity

B, S, E, H, D = 16, 1024, 512, 8, 64
NCORES = 8
BPC = B // NCORES  # batches per core
SCALE = D**-0.5
BF16 = mybir.dt.bfloat16
F32 = mybir.dt.float32
NPBF16 = ml_dtypes.bfloat16

_cache = {}

NU = BPC * 4  # pipeline units: (batch, head-pair)


def _build(bo_nonzero: bool, bv_nonzero: bool = True, knobs=None):
    import os
    knobs = knobs or {}
    WARM = int(knobs.get("warm", os.environ.get("K_WARM", 8)))
    SCMOD = int(knobs.get("scmod", os.environ.get("K_SCMOD", 2)))
    SCRES = int(knobs.get("scres", os.environ.get("K_SCRES", 1)))
    EXPQ = int(knobs.get("expq", os.environ.get("K_EXPQ", 4)))
    NFE = int(knobs.get("nfe", os.environ.get("K_NFE", 0)))  # heads on gpsimd fast-exp
    assert EXPQ == 3 or 8 % EXPQ == 0, "EXPQ must be 3 or divide 8"
    """Build + compile the per-core Bass graph (same graph on all 8 cores)."""
    nc = bacc.Bacc("TRN2", target_bir_lowering=False, debug=False, num_devices=NCORES)

    xT_d = nc.dram_tensor("xT", [BPC, E, S], BF16, kind="ExternalInput").ap()
    maskT_d = nc.dram_tensor("maskT", [BPC, S, S], BF16, kind="ExternalInput").ap()
    wq_d = nc.dram_tensor("wqT", [E, E], BF16, kind="ExternalInput").ap()
    wk_d = nc.dram_tensor("wkT", [E, E], BF16, kind="ExternalInput").ap()
    wv_d = nc.dram_tensor("wvT", [E, E], BF16, kind="ExternalInput").ap()
    wo_d = nc.dram_tensor("woT", [E, E], BF16, kind="ExternalInput").ap()
    bqk_d = nc.dram_tensor("bqk", [128, 8], F32, kind="ExternalInput").ap()
    bv_d = nc.dram_tensor("bv", [E], F32, kind="ExternalInput").ap()
    bo_d = nc.dram_tensor("bo", [E], F32, kind="ExternalInput").ap()
    out_d = nc.dram_tensor("out", [BPC, S, E], F32, kind="ExternalOutput").ap()

    mult = mybir.AluOpType.mult
    add = mybir.AluOpType.add
    EXP = mybir.ActivationFunctionType.Exp
    IDENT = mybir.ActivationFunctionType.Identity
    COPY = mybir.ActivationFunctionType.Copy

    with tile.TileContext(nc) as tc, ExitStack() as ctx:
        singles = ctx.enter_context(tc.tile_pool(name="singles", bufs=1))
        xtp = ctx.enter_context(tc.tile_pool(name="xt", bufs=BPC))
        maskp = ctx.enter_context(tc.tile_pool(name="mask", bufs=BPC))
        qkp = ctx.enter_context(tc.tile_pool(name="qk", bufs=6))
        vp = ctx.enter_context(tc.tile_pool(name="v", bufs=8 * BPC))
        megap = ctx.enter_context(tc.tile_pool(name="mega", bufs=4))
        ctxp = ctx.enter_context(tc.tile_pool(name="ctx", bufs=8 * BPC))
        ctxTp = ctx.enter_context(tc.tile_pool(name="ctxT", bufs=4 * BPC))
        outp = ctx.enter_context(tc.tile_pool(name="outs", bufs=2))
        rcp = ctx.enter_context(tc.tile_pool(name="rc", bufs=8))
        scp = ctx.enter_context(tc.tile_pool(name="sc", bufs=2, space="PSUM"))
        pjp = ctx.enter_context(tc.tile_pool(name="pj", bufs=2, space="PSUM"))
        mmp = ctx.enter_context(tc.tile_pool(name="mm", bufs=2, space="PSUM"))

        # ---- constants ----
        w_sb = {}

        def load_w(name, d, eng=None):
            t = singles.tile([128, 4 * E], BF16, tag=f"w{name}", name=f"w{name}")
            ov = t[:].rearrange("p (c f) -> p c f", c=4)
            iv = d.rearrange("(c p) f -> p c f", p=128)
            (eng or nc.sync).dma_start(out=ov, in_=iv)
            w_sb[name] = t

        bqk_sb = singles.tile([128, 8], F32, tag="bqk")
        nc.sync.dma_start(out=bqk_sb[:], in_=bqk_d[:])
        bv_sb = singles.tile([128, E], F32, tag="bv")
        bo_sb = None
        ident = singles.tile([128, 128], BF16, tag="ident")
        make_identity(nc, ident[:])
        warm_in = singles.tile([128, 512], BF16, tag="warm")
        nc.gpsimd.memset(warm_in[:], 0.0)
        warm_ps = mmp.tile([128, 512], F32, tag="mm", name="warmps")
        for wi in range(WARM):
            nc.tensor.matmul(
                warm_ps[:], lhsT=ident[:], rhs=warm_in[:],
                start=True, stop=True,
            )

        # ---- pipeline state ----
        xt = {}      # b -> [4 tiles]
        masks = {}   # b -> [8 tiles]
        qk = {}      # (b, 'q'|'k', j) -> tile [128, S]
        v_sb = {}    # b -> [8 tiles]
        mega = {}    # (u, hh) -> tile
        ctx_sb = {}  # b -> [8 tiles]
        ctxT = {}    # (b, j) -> tile

        def dma_in_x(b, half=None):
            if b in xt:
                t = xt[b]
            else:
                t = xtp.tile([128, 4 * S], BF16, tag="xt", name=f"xt{b}")
                xt[b] = t
            ov = t[:].rearrange("p (e s) -> p e s", e=4)
            iv = xT_d[b].rearrange("(e p) s -> p e s", p=128)
            if half in (None, 0):
                nc.sync.dma_start(out=ov[:, 0:2], in_=iv[:, 0:2])
            if half in (None, 1):
                nc.sync.dma_start(out=ov[:, 2:4], in_=iv[:, 2:4])

        def dma_in_mask(b, pieces=((0, 4), (4, 8))):
            if b in masks:
                t = masks[b]
            else:
                t = maskp.tile([128, 8 * S], BF16, tag="mask", name=f"mask{b}")
                masks[b] = t
            ov = t[:].rearrange("p (sk sq) -> p sk sq", sk=8)
            iv = maskT_d[b].rearrange("(sk p) sq -> p sk sq", p=128)
            for lo, hi in pieces:
                nc.sync.dma_start(out=ov[:, lo:hi], in_=iv[:, lo:hi])

        def dma_in(b):
            dma_in_x(b)
            dma_in_mask(b)

        def proj_qk(b, j):
            for ti, tname in enumerate(("q", "k")):
                t = qkp.tile([128, S], BF16, tag="qk", name=f"qk{b}_{tname}{j}")
                col = ti * 4 + j
                for sh in range(2):
                    ps = pjp.tile([128, 512], F32, tag="pj", name=f"pqk{b}{j}{tname}{sh}")
                    for e in range(4):
                        nc.tensor.matmul(
                            ps[:],
                            lhsT=w_sb[tname][:, e * E + j * 128 : e * E + (j + 1) * 128],
                            rhs=xt[b][:, e * S + sh * 512 : e * S + (sh + 1) * 512],
                            start=(e == 0),
                            stop=(e == 3),
                        )
                    nc.scalar.activation(
                        t[:, sh * 512 : (sh + 1) * 512], ps[:], IDENT,
                        bias=bqk_sb[:, col : col + 1], scale=1.0,
                    )
                qk[(b, tname, j)] = t

        def proj_v(b):
            vs = []
            for s in range(8):
                ps = mmp.tile([128, 512], F32, tag="mm", name=f"pv{b}_{s}")
                for e in range(4):
                    nc.tensor.matmul(
                        ps[:],
                        lhsT=xt[b][:, e * S + s * 128 : e * S + (s + 1) * 128],
                        rhs=w_sb["v"][:, e * E : (e + 1) * E],
                        start=(e == 0),
                        stop=(e == 3),
                    )
                vt = vp.tile([128, 8 * 65], BF16, tag="v", name=f"v{b}_{s}")
                vv = vt[:].rearrange("p (h c) -> p h c", h=8)
                pv = ps[:].rearrange("p (h c) -> p h c", h=8)
                nc.scalar.activation(vv[:, :, 0:64], pv[:, :, :], COPY)
                nc.gpsimd.memset(vv[:, :, 64:65], 1.0)
                vs.append(vt)
            v_sb[b] = vs

        def attn_a(u, bsteps, bsteps2=None):
            """scores + fused mask-mul + exp for both heads of unit u.
            Pops PE-side backlog work (bsteps) between score tiles so the
            PE feeds the vector engine continuously (engines are in-order)."""
            b, j = divmod(u, 4)
            kT = qk[(b, "k", j)]
            qT = qk[(b, "q", j)]
            slot = 0
            for hh in range(2):
                if hh == 1 and bsteps2:
                    bsteps = bsteps + bsteps2
                mg = megap.tile([128, 8 * S], BF16, tag="mega", name=f"mega{u}_{hh}")
                for sk in range(8):
                    ps = scp.tile([128, S], F32, tag="sc", name=f"sc{u}{hh}{sk}")
                    for sh in range(2):
                        nc.tensor.matmul(
                            ps[:, sh * 512 : (sh + 1) * 512],
                            lhsT=kT[hh * 64 : hh * 64 + 64, sk * 128 : sk * 128 + 128],
                            rhs=qT[hh * 64 : hh * 64 + 64, sh * 512 : (sh + 1) * 512],
                            start=True,
                            stop=True,
                        )
                    nc.vector.scalar_tensor_tensor(
                        out=mg[:, sk * S : (sk + 1) * S],
                        in0=ps[:],
                        scalar=1.0,
                        in1=masks[b][:, sk * S : (sk + 1) * S],
                        op0=mult,
                        op1=mult,
                    )
                    slots_left = 16 - slot
                    n = (len(bsteps) + slots_left - 1) // slots_left if bsteps else 0
                    for _ in range(n):
                        if bsteps:
                            bsteps.pop(0)()
                    slot += 1
                if (u, hh) in fe_heads:
                    # Schraudolph fast-exp on the idle GpSimd engine: the
                    # int16 bits of round(x*128/ln2 + (127*128-7.42)) ARE the
                    # bf16 representation of exp(x) (~1.8% rms) -- one
                    # in-place pass, offloading this head's exp from ACT.
                    mgi = mg[:].bitcast(mybir.dt.int16)
                    for qq in range(4):
                        sl = slice(qq * 2 * S, (qq + 1) * 2 * S)
                        nc.gpsimd.tensor_scalar(
                            mgi[:, sl], mg[:, sl], 184.66496, 16248.577, mult, add
                        )
                elif u == NU - 1:
                    for qq in range(4):
                        nc.scalar.activation(
                            mg[:, qq * 2 * S : (qq + 1) * 2 * S],
                            mg[:, qq * 2 * S : (qq + 1) * 2 * S], EXP,
                        )
                elif EXPQ == 3:
                    for lo, hi in ((0, 2), (2, 4), (4, 8)):
                        nc.scalar.activation(
                            mg[:, lo * S : hi * S], mg[:, lo * S : hi * S], EXP
                        )
                else:
                    w = 8 // EXPQ
                    for qq in range(EXPQ):
                        nc.scalar.activation(
                            mg[:, qq * w * S : (qq + 1) * w * S],
                            mg[:, qq * w * S : (qq + 1) * w * S], EXP,
                        )
                mega[(u, hh)] = mg
            for st in bsteps:
                st()

        def attn_b_steps(u):
            """ctx matmul + normalize (8 steps) then transposes (4 steps)."""
            b, j = divmod(u, 4)
            steps = []

            def mk_ctx(sq):
                def step():
                    if j == 0 and sq == 0 and b not in ctx_sb:
                        ctx_sb[b] = [
                            ctxp.tile([128, E], BF16, tag="ctx", name=f"ctx{b}_{i}")
                            for i in range(8)
                        ]
                    pc = mmp.tile([128, 130], F32, tag="mm", name=f"pc{u}_{sq}")
                    for hh in range(2):
                        h = 2 * j + hh
                        mg = mega[(u, hh)]
                        for sk in range(8):
                            nc.tensor.matmul(
                                pc[:, hh * 65 : hh * 65 + 65],
                                lhsT=mg[:, sk * S + sq * 128 : sk * S + sq * 128 + 128],
                                rhs=v_sb[b][sk][:, h * 65 : h * 65 + 65],
                                start=(sk == 0),
                                stop=(sk == 7),
                            )
                    rc = rcp.tile([128, 2], F32, tag="rc", name=f"rc{u}_{sq}")
                    pcv = pc[:].rearrange("p (h c) -> p h c", h=2)
                    nc.vector.reciprocal(rc[:], pcv[:, :, 64])
                    for hh in range(2):
                        h = 2 * j + hh
                        if bv_nonzero or sq % SCMOD != SCRES:
                            nc.vector.scalar_tensor_tensor(
                                out=ctx_sb[b][sq][:, h * 64 : h * 64 + 64],
                                in0=pc[:, hh * 65 : hh * 65 + 64],
                                scalar=rc[:, hh : hh + 1],
                                in1=bv_sb[:, h * 64 : h * 64 + 64],
                                op0=mult,
                                op1=add,
                            )
                        else:
                            nc.scalar.activation(
                                ctx_sb[b][sq][:, h * 64 : h * 64 + 64],
                                pc[:, hh * 65 : hh * 65 + 64],
                                COPY,
                                scale=rc[:, hh : hh + 1],
                            )
                return step

            for sq in range(8):
                steps.append(mk_ctx(sq))

            cT = ctxTp.tile([128, S], BF16, tag="ctxT", name=f"ctxT{b}_{j}")
            ctxT[(b, j)] = cT

            def mk_tr(sq4):
                def step():
                    pt = mmp.tile([128, 512], BF16, tag="mm", name=f"pt{u}_{sq4}")
                    for hh in range(2):
                        for sqi in range(4):
                            sq = sq4 * 4 + sqi
                            nc.tensor.transpose(
                                out=pt[hh * 64 : hh * 64 + 64, sqi * 128 : sqi * 128 + 128],
                                in_=ctx_sb[b][sq][:, (2 * j + hh) * 64 : (2 * j + hh) * 64 + 64],
                                identity=ident[:],
                            )
                    nc.scalar.activation(
                        cT[:, sq4 * 512 : (sq4 + 1) * 512], pt[:], COPY
                    )
                return step

            for sq4 in range(2):
                steps.append(mk_tr(sq4))
            return steps

        def proj_qk_steps(b, j, split=False, dve_copy=False):
            steps = []
            for ti, tname in enumerate(("q", "k")):
                t = qkp.tile([128, S], BF16, tag="qk", name=f"qk{b}_{tname}{j}")
                qk[(b, tname, j)] = t
                col = ti * 4 + j

                def mk(tname=tname, t=t, col=col, sh=0):
                    def step():
                        ps = pjp.tile(
                            [128, 512], F32, tag="pj", name=f"pqk{b}{j}{tname}{sh}"
                        )
                        for e in range(4):
                            nc.tensor.matmul(
                                ps[:],
                                lhsT=w_sb[tname][:, e * E + j * 128 : e * E + (j + 1) * 128],
                                rhs=xt[b][:, e * S + sh * 512 : e * S + (sh + 1) * 512],
                                start=(e == 0),
                                stop=(e == 3),
                            )
                        nc.scalar.activation(
                            t[:, sh * 512 : (sh + 1) * 512], ps[:], IDENT,
                            bias=bqk_sb[:, col : col + 1], scale=1.0,
                        )
                    return step

                if split:
                    steps.append(mk(sh=0))
                    steps.append(mk(sh=1))
                else:
                    def mkf(tname=tname, t=t, col=col):
                        def step():
                            ps = scp.tile(
                                [128, S], F32, tag="sc", name=f"pqk{b}{j}{tname}"
                            )
                            for sh in range(2):
                                for e in range(4):
                                    nc.tensor.matmul(
                                        ps[:, sh * 512 : (sh + 1) * 512],
                                        lhsT=w_sb[tname][:, e * E + j * 128 : e * E + (j + 1) * 128],
                                        rhs=xt[b][:, e * S + sh * 512 : e * S + (sh + 1) * 512],
                                        start=(e == 0),
                                        stop=(e == 3),
                                    )
                            nc.scalar.activation(
                                t[:], ps[:], IDENT,
                                bias=bqk_sb[:, col : col + 1], scale=1.0,
                            )
                        return step

                    steps.append(mkf())
            return steps

        def proj_v_steps(b):
            v_sb[b] = [None] * 8
            steps = []

            def mk(s):
                def step():
                    ps = mmp.tile([128, 512], F32, tag="mm", name=f"pv{b}_{s}")
                    for e in range(4):
                        nc.tensor.matmul(
                            ps[:],
                            lhsT=xt[b][:, e * S + s * 128 : e * S + (s + 1) * 128],
                            rhs=w_sb["v"][:, e * E : (e + 1) * E],
                            start=(e == 0),
                            stop=(e == 3),
                        )
                    vt = vp.tile([128, 8 * 65], BF16, tag="v", name=f"v{b}_{s}")
                    vv = vt[:].rearrange("p (h c) -> p h c", h=8)
                    pv = ps[:].rearrange("p (h c) -> p h c", h=8)
                    nc.scalar.activation(vv[:, :, 0:64], pv[:, :, :], COPY)
                    nc.gpsimd.memset(vv[:, :, 64:65], 1.0)
                    v_sb[b][s] = vt
                return step

            for s in range(8):
                steps.append(mk(s))
            return steps

        def tail_ctx_steps(u, hh):
            """per-head ctx+normalize steps ([128,65] psum tiles) for the last unit."""
            b, j = divmod(u, 4)
            steps = []

            def mk(sq):
                def step():
                    if j == 0 and b not in ctx_sb:
                        ctx_sb[b] = [
                            ctxp.tile([128, E], BF16, tag="ctx", name=f"ctx{b}_{i}")
                            for i in range(8)
                        ]
                    h = 2 * j + hh
                    pc = mmp.tile([128, 65], F32, tag="mm", name=f"tc{u}_{hh}_{sq}")
                    mg = mega[(u, hh)]
                    for sk in range(8):
                        nc.tensor.matmul(
                            pc[:],
                            lhsT=mg[:, sk * S + sq * 128 : sk * S + sq * 128 + 128],
                            rhs=v_sb[b][sk][:, h * 65 : h * 65 + 65],
                            start=(sk == 0),
                            stop=(sk == 7),
                        )
                    rc = rcp.tile([128, 1], F32, tag="rc", name=f"trc{u}_{hh}_{sq}")
                    nc.vector.reciprocal(rc[:], pc[:, 64:65])
                    nc.vector.scalar_tensor_tensor(
                        out=ctx_sb[b][sq][:, h * 64 : h * 64 + 64],
                        in0=pc[:, 0:64],
                        scalar=rc[:, 0:1],
                        in1=bv_sb[:, h * 64 : h * 64 + 64],
                        op0=mult,
                        op1=add,
                    )
                return step

            for sq in range(8):
                steps.append(mk(sq))
            return steps

        def outproj_steps(b, half):
            steps = []
            state = {}

            def mk(si):
                def step():
                    s = half * 4 + si
                    if si % 2 == 0:
                        state["ou"] = outp.tile(
                            [128, 2 * 512], F32, tag="outs", name=f"ou{b}_{s}"
                        )
                    ou = state["ou"]
                    po = mmp.tile([128, 512], F32, tag="mm", name=f"po{b}_{s}")
                    for j in range(4):
                        nc.tensor.matmul(
                            po[:],
                            lhsT=ctxT[(b, j)][:, s * 128 : (s + 1) * 128],
                            rhs=w_sb["o"][:, j * E : (j + 1) * E],
                            start=(j == 0),
                            stop=(j == 3),
                        )
                    osl = ou[:, (si % 2) * 512 : (si % 2 + 1) * 512]
                    if bo_nonzero:
                        nc.vector.scalar_tensor_tensor(
                            out=osl, in0=po[:], scalar=1.0, in1=bo_sb[:],
                            op0=mult, op1=add,
                        )
                    else:
                        nc.scalar.activation(osl, po[:], COPY)
                    if si % 2 == 1:
                        qs = half * 4 + si - 1
                        nc.sync.dma_start(
                            out=out_d[b, qs * 128 : (qs + 1) * 128, :],
                            in_=ou[:, 0:512],
                        )
                        nc.sync.dma_start(
                            out=out_d[b, (qs + 1) * 128 : (qs + 2) * 128, :],
                            in_=ou[:, 512:1024],
                        )
                return step

            for si in range(4):
                steps.append(mk(si))
            return steps

        # ---- pipelined emission ----
        load_w("q", wq_d)
        dma_in_x(0)
        load_w("k", wk_d)
        dma_in_mask(0, pieces=((0, 1), (1, 2), (2, 4), (4, 8)))
        load_w("v", wv_d)
        load_w("o", wo_d)
        nc.sync.dma_start(
            out=bv_sb[:],
            in_=bass.AP(tensor=bv_d.tensor, offset=bv_d.offset, ap=[[0, 128]] + bv_d.ap),
        )
        if bo_nonzero:
            bo_sb = singles.tile([128, E], F32, tag="bo")
            nc.sync.dma_start(
                out=bo_sb[:],
                in_=bass.AP(
                    tensor=bo_d.tensor, offset=bo_d.offset, ap=[[0, 128]] + bo_d.ap
                ),
            )
        fe_all = [(2, 1), (5, 1), (3, 0), (4, 0), (1, 1), (6, 0)]
        fe_heads = set(fe_all[:NFE])

        for st in proj_qk_steps(0, 0, split=True):
            st()
        for u in range(NU):
            b, j = divmod(u, 4)
            if u == 1 and BPC > 1:
                dma_in(1)
            bsteps = []
            if u + 1 < NU:
                nb, nj = divmod(u + 1, 4)
                bsteps += proj_qk_steps(nb, nj, split=True)
            if j == 0:
                bsteps += proj_v_steps(b)
            if u >= 1:
                bsteps += attn_b_steps(u - 1)
            if u == 5:
                bsteps += outproj_steps(0, 0)
            if u == 6:
                bsteps += outproj_steps(0, 1)
            bsteps2 = tail_ctx_steps(u, 0) if u == NU - 1 else None
            attn_a(u, bsteps, bsteps2)
        # tail drain: ctx for the second head, interleaved with the last
        # pair's transposes and the final out-projection.
        hh1_steps = tail_ctx_steps(NU - 1, 1)
        b_l, j_l = divmod(NU - 1, 4)
        cT_l = ctxTp.tile([128, S], BF16, tag="ctxT", name=f"ctxT{b_l}_{j_l}")
        ctxT[(b_l, j_l)] = cT_l

        def tr_step(sq4):
            pt = mmp.tile([128, 512], BF16, tag="mm", name=f"ptL_{sq4}")
            for hh in range(2):
                for sqi in range(4):
                    sq = sq4 * 4 + sqi
                    nc.tensor.transpose(
                        out=pt[hh * 64 : hh * 64 + 64, sqi * 128 : sqi * 128 + 128],
                        in_=ctx_sb[b_l][sq][:, (2 * j_l + hh) * 64 : (2 * j_l + hh) * 64 + 64],
                        identity=ident[:],
                    )
            nc.vector.tensor_scalar_mul(
                cT_l[:, sq4 * 512 : (sq4 + 1) * 512], pt[:], 1.0
            )

        for st in hh1_steps:  # hh1 ctx per sq-chunk
            st()
        lb = BPC - 1
        for sq4 in range(2):
            tr_step(sq4)  # transposes + ctxT copy for sq-quad sq4
            for qq in range(2):
                sq2 = sq4 * 2 + qq
                ou = outp.tile([128, 2 * 512], F32, tag="outs", name=f"ouL_{sq2}")
                for si in range(2):
                    s = sq2 * 2 + si
                    po = mmp.tile([128, 512], F32, tag="mm", name=f"poL_{s}")
                    for j in range(4):
                        nc.tensor.matmul(
                            po[:],
                            lhsT=ctxT[(lb, j)][:, s * 128 : (s + 1) * 128],
                            rhs=w_sb["o"][:, j * E : (j + 1) * E],
                            start=(j == 0),
                            stop=(j == 3),
                        )
                    osl = ou[:, si * 512 : (si + 1) * 512]
                    if bo_nonzero:
                        nc.vector.scalar_tensor_tensor(
                            out=osl, in0=po[:], scalar=1.0, in1=bo_sb[:],
                            op0=mult, op1=add,
                        )
                    else:
                        nc.scalar.activation(osl, po[:], COPY)
                nc.sync.dma_start(
                    out=out_d[lb, sq2 * 256 : sq2 * 256 + 128, :], in_=ou[:, 0:512]
                )
                nc.sync.dma_start(
                    out=out_d[lb, sq2 * 256 + 128 : (sq2 + 1) * 256, :],
                    in_=ou[:, 512:1024],
                )

    nc.compile()
    return nc


def _prep(x, adj_matrix, bond_matrix, Wq, bq, Wk, bk, Wv, bv, Wo, bo):
    """Host-side layout prep. Returns per-core input maps."""
    x = np.asarray(x, np.float32)
    mask = np.asarray(adj_matrix, np.float32) + np.asarray(bond_matrix, np.float32)
    xT = np.ascontiguousarray(x.transpose(0, 2, 1)).astype(NPBF16)
    maskT = np.ascontiguousarray(mask.transpose(0, 2, 1)).astype(NPBF16)
    wqT = np.ascontiguousarray(np.asarray(Wq, np.float32).T * SCALE).astype(NPBF16)
    wkT = np.ascontiguousarray(np.asarray(Wk, np.float32).T).astype(NPBF16)
    wvT = np.ascontiguousarray(np.asarray(Wv, np.float32).T).astype(NPBF16)
    woT = np.ascontiguousarray(np.asarray(Wo, np.float32).T).astype(NPBF16)
    bqs = np.asarray(bq, np.float32) * SCALE
    bkf = np.asarray(bk, np.float32)
    # [128, 8]: cols 0-3 = bq chunks, 4-7 = bk chunks (chunk c = f in [128c,128c+128))
    bqk = np.concatenate(
        [bqs.reshape(4, 128).T, bkf.reshape(4, 128).T], axis=1
    ).astype(np.float32)
    bqk = np.ascontiguousarray(bqk)
    bvf = np.ascontiguousarray(np.asarray(bv, np.float32))
    bof = np.ascontiguousarray(np.asarray(bo, np.float32))

    in_maps = []
    for c in range(NCORES):
        sl = slice(c * BPC, (c + 1) * BPC)
        in_maps.append(
            {
                "xT": np.ascontiguousarray(xT[sl]),
                "maskT": np.ascontiguousarray(maskT[sl]),
                "wqT": wqT,
                "wkT": wkT,
                "wvT": wvT,
                "woT": woT,
                "bqk": bqk,
                "bv": bvf,
                "bo": bof,
            }
        )
    return in_maps, bool(np.any(bof))


def kernel(
    x,
    adj_matrix,
    bond_matrix,
    Wq,
    bq,
    Wk,
    bk,
    Wv,
    bv,
    Wo,
    bo,
    seq_len,
    _trace=False,
):
    in_maps, bo_nonzero = _prep(
        x, adj_matrix, bond_matrix, Wq, bq, Wk, bk, Wv, bv, Wo, bo
    )
    bv_nonzero = bool(np.any(np.asarray(bv)))
    key = ("k", bo_nonzero, bv_nonzero)
    if key not in _cache:
        _cache[key] = _build(bo_nonzero, bv_nonzero)
    nc = _cache[key]
    res = run_bass_kernel_spmd(
        nc, in_maps, core_ids=list(range(NCORES)), trace=_trace
    )
    out = np.concatenate([r["out"] for r in res.results], axis=0).astype(np.float32)
    if _trace:
        kernel._last_exec_time_ns = res.exec_time_ns
        kernel._last_results = res
    return out

